# revision 53
# baseline (speedup 1.0000x reference)
"""Trainium2 Bass kernel for a pre-LN transformer encoder layer.

Sharding: data-parallel over batch. B=8 batch elements -> 8 NeuronCores,
one full [L=1024, D=1024] encoder layer per core. No collectives.

Per-core dataflow (q = token index, d = feature index, k = key index):
  x [q,d] --LN1--> x1 [q,d] --PE transpose--> x1T [d,q] (bf16)
  V natural [k,d] (+ones col per head)  = matmul(lhsT=x1T tile, rhs=Wv rows)
  QT, KT [d,q]                          = matmul(lhsT=W col block, rhs=x1T)
  per head pair (chunk-major): ST [k,q] psum (row-packed across the two
            64-row head groups); exp via ACT for even k-tiles and a
            Schraudolph int16-bit-trick on DVE for odd k-tiles (splits the
            PSUM-drain load across two engines); PV' accumulates
            [attnT | Z] over k tiles (ones-column trick); 1/Z via DVE
            reciprocal of the Z row + gpsimd partition_broadcast; DVE
            multiply writes attnT [d,q] directly.
  attnproj [q,d] = matmul(lhsT=attnT tile, rhs=Wo rows) + bo via a K=1
            ones-row matmul folded into the chain; x2 = x + proj.
  LN2 -> x2n -> transpose -> x2nT [d,q]
  FFN in two halves of 16 f-tiles: hT [f,q] = matmul(lhsT=W1 col block,
            rhs=x2nT), ReLU+b1 fused in ACT; FFN2 accumulates K=2048
            chains in PSUM (plus b2 ones-row matmul in half 0), so only
            two DVE adds per output chunk.

Input x is DMA'd first on both HWDGE queues (sync+scalar) before any
weight prefetch traffic; a short stream of dummy K=1 matmuls warms the
PE clock (HAM) before real work arrives. Attention runs chunk-major so
the proj/LN2/FFN pipeline for the first half overlaps the softmax of
the second half. All matmul operands are bf16; stats/softmax/residual
arithmetic stays fp32.
"""

import numpy as np

import concourse.bass as bass
import concourse.tile as tile
from concourse import bacc, mybir
from concourse.bass import ds, ts
from concourse.masks import make_identity

B = 8
L = 1024
D = 1024
H = 16
DK = 64
F = 4096
EPS = 1e-6
NEG_INF = 1.0e9
P = 128
NQ = L // P            # 8 token tiles
ND = D // P            # 8 model-dim tiles
NF = F // P            # 32 ffn-dim tiles
CH = 512               # matmul moving free dim (one PSUM bank of fp32)
NCH = L // CH          # 2 chunks of tokens
QPC = CH // P          # 4 q-tiles per chunk
HPC = CH // DK         # 8 heads per 512-wide projection chunk
NHALF = NF // 2        # 16 f-tiles per FFN half

FP32 = mybir.dt.float32
MMD = mybir.dt.bfloat16   # matmul operand dtype
I16 = mybir.dt.int16
AF = mybir.ActivationFunctionType
OP = mybir.AluOpType

# Schraudolph exp: bf16 bits of exp(x) ~= (2^23/ln2 * x + B32) / 65536
SCHRA_A = 12102203.161561485 / 65536.0     # per unit of exp argument
SCHRA_B = 16248.33                          # (127*2^23 - 486408)/65536


def build_nc():
    nc = bacc.Bacc("TRN2", target_bir_lowering=False, num_swdge_queues=4)

    xd = nc.dram_tensor("x", [L, D], FP32, kind="ExternalInput")
    maskd = nc.dram_tensor("e_mask", [1, L], mybir.dt.int32, kind="ExternalInput")
    ln1_g = nc.dram_tensor("ln1_g", [D], FP32, kind="ExternalInput")
    ln1_b = nc.dram_tensor("ln1_b", [D], FP32, kind="ExternalInput")
    wq = nc.dram_tensor("Wq", [D, D], FP32, kind="ExternalInput")
    bq = nc.dram_tensor("bq", [D], FP32, kind="ExternalInput")
    wk = nc.dram_tensor("Wk", [D, D], FP32, kind="ExternalInput")
    bk = nc.dram_tensor("bk", [D], FP32, kind="ExternalInput")
    wv = nc.dram_tensor("Wv", [D, D], FP32, kind="ExternalInput")
    bv = nc.dram_tensor("bv", [D], FP32, kind="ExternalInput")
    wo = nc.dram_tensor("Wo", [D, D], FP32, kind="ExternalInput")
    bo = nc.dram_tensor("bo", [D], FP32, kind="ExternalInput")
    ln2_g = nc.dram_tensor("ln2_g", [D], FP32, kind="ExternalInput")
    ln2_b = nc.dram_tensor("ln2_b", [D], FP32, kind="ExternalInput")
    w1 = nc.dram_tensor("W1", [D, F], FP32, kind="ExternalInput")
    b1 = nc.dram_tensor("b1", [F], FP32, kind="ExternalInput")
    w2 = nc.dram_tensor("W2", [F, D], FP32, kind="ExternalInput")
    b2 = nc.dram_tensor("b2", [D], FP32, kind="ExternalInput")
    outd = nc.dram_tensor("out", [L, D], FP32, kind="ExternalOutput")

    with tile.TileContext(nc) as tc:
        singles = tc.alloc_tile_pool(name="singles", bufs=1)
        big = tc.alloc_tile_pool(name="big", bufs=1)
        # single PSUM pool for the whole kernel: no pool-release barriers.
        # 4 (mm chains) + 2 (attention PV) + 2 (transposes) = 8 banks.
        psum = tc.alloc_tile_pool(name="psum", bufs=1, space="PSUM")

        def psum_mm():
            return psum.tile([P, CH], FP32, tag="mm", name="ps_mm", bufs=4)

        def big_tiles(shape, tagp, namep, dt=FP32):
            return [
                big.tile(shape, dt, tag=f"{tagp}{i}", name=f"{namep}{i}", bufs=1)
                for i in range(NQ)
            ]

        # weight pools allocated before ph1 so pool release stays LIFO;
        # their tiles/DMAs are emitted after the x loads below.
        ph2v = tc.alloc_tile_pool(name="ph2v", bufs=1)
        ph4w = tc.alloc_tile_pool(name="ph4w", bufs=1)

        # ---------- phase 0: input DMAs first, then PE warmup ----------
        ph1 = tc.alloc_tile_pool(name="ph1", bufs=1)
        x_in = [
            ph1.tile([P, D], FP32, tag=f"x_in{qt}", name=f"x_in{qt}", bufs=1)
            for qt in range(NQ)
        ]
        for qt in range(NQ):
            eng = nc.sync if qt % 2 == 0 else nc.scalar
            eng.dma_start(out=x_in[qt], in_=xd.ap()[ts(qt, P), :])

        warm = singles.tile([1, CH], MMD, name="warm")
        nc.vector.memset(warm, 0.0)
        wps = psum_mm()
        for _ in range(20):
            nc.tensor.matmul(wps[0:1, :], warm[0:1, 0:1], warm[0:1, :],
                             start=True, stop=True)

        ident = singles.tile([P, P], MMD, name="ident")
        make_identity(nc, ident)
        eps_t = singles.tile([P, 1], FP32, name="eps_t")
        nc.vector.memset(eps_t, EPS)
        ones_h = singles.tile([P, H, 1], FP32, name="ones_h")
        nc.vector.memset(ones_h, 1.0)
        ones_row = singles.tile([1, P], MMD, name="ones_row")
        nc.vector.memset(ones_row, 1.0)
        bo_row = singles.tile([1, D], MMD, name="bo_row")
        nc.gpsimd.dma_start(out=bo_row, in_=bo.ap().unsqueeze(0))
        b2_row = singles.tile([1, D], MMD, name="b2_row")
        nc.gpsimd.dma_start(out=b2_row, in_=b2.ap().unsqueeze(0))
        bv_row = singles.tile([1, D], MMD, name="bv_row")
        nc.gpsimd.dma_start(out=bv_row, in_=bv.ap().unsqueeze(0))

        def bcast_load(pool, dram_vec, n, tag):
            """replicate a [n] DRAM vector across all 128 partitions."""
            t = pool.tile([P, n], FP32, tag=tag, name=tag, bufs=1)
            src = bass.AP(
                tensor=dram_vec.tensor,
                offset=dram_vec.offset,
                ap=[[0, P], [1, n]],
            )
            nc.sync.dma_start(out=t, in_=src)
            return t

        def col_load(dram_vec, ntiles, name):
            """[ntiles*128] DRAM vector -> [128, ntiles], col t = v[t*128:+128]."""
            t = singles.tile([P, ntiles], FP32, name=name)
            nc.gpsimd.dma_start(out=t, in_=dram_vec.rearrange("(t p) -> p t", p=P))
            return t

        g1_c = col_load(ln1_g.ap(), ND, "g1_c")
        b1ln_c = col_load(ln1_b.ap(), ND, "b1ln_c")
        g2_c = col_load(ln2_g.ap(), ND, "g2_c")
        b2ln_c = col_load(ln2_b.ap(), ND, "b2ln_c")
        bq_c = col_load(bq.ap(), ND, "bq_c")
        bk_c = col_load(bk.ap(), ND, "bk_c")
        b1_c = col_load(b1.ap(), NF, "b1_c")

        # additive attention-mask bias per key position: (mask-1)*NEG_INF
        mask_i = singles.tile([P, NQ], mybir.dt.int32, name="mask_i")
        nc.gpsimd.dma_start(out=mask_i, in_=maskd.ap()[0].rearrange("(t p) -> p t", p=P))
        mask_f = singles.tile([P, NQ], FP32, name="mask_f")
        nc.vector.tensor_copy(out=mask_f, in_=mask_i)
        ebias = singles.tile([P, NQ], FP32, name="ebias")
        nc.vector.tensor_scalar(
            out=ebias, in0=mask_f, scalar1=1.0, scalar2=NEG_INF,
            op0=OP.subtract, op1=OP.mult,
        )
        # Schraudolph bias column per k-tile: ebias*184.66 + B16
        eb16 = singles.tile([P, NQ], FP32, name="eb16")
        nc.vector.tensor_scalar(
            out=eb16, in0=ebias, scalar1=SCHRA_A,
            scalar2=SCHRA_B, op0=OP.mult, op1=OP.add,
        )

        def layer_norm_tile(pool, x_t, use_act=False):
            stats = pool.tile([P, 2, 6], FP32, tag="ln_stats", name="ln_stats")
            xr = x_t.rearrange("p (s c) -> p s c", s=2)
            for s in range(2):
                nc.vector.bn_stats(out=stats[:, s, :], in_=xr[:, s, :])
            mv = pool.tile([P, 2], FP32, tag="ln_mv", name="ln_mv")
            nc.vector.bn_aggr(out=mv, in_=stats)
            rstd = pool.tile([P, 1], FP32, tag="ln_rstd", name="ln_rstd")
            nc.scalar.activation(out=rstd, in_=mv[:, 1:2], func=AF.Sqrt,
                                 bias=eps_t, scale=1.0)
            nc.vector.reciprocal(out=rstd, in_=rstd)
            xn = pool.tile([P, D], MMD, tag="ln_out", name="ln_out")
            if use_act:
                # apply on the (early-idle) ACT engine: (x - mu) * rstd
                # == x * rstd + (-mu * rstd)
                nmr = pool.tile([P, 1], FP32, tag="ln_nmr", name="ln_nmr")
                nc.vector.tensor_scalar(
                    out=nmr, in0=mv[:, 0:1], scalar1=rstd, scalar2=-1.0,
                    op0=OP.mult, op1=OP.mult,
                )
                nc.scalar.activation(out=xn, in_=x_t, func=AF.Identity,
                                     bias=nmr, scale=rstd)
            else:
                nc.vector.tensor_scalar(
                    out=xn, in0=x_t, scalar1=mv[:, 0:1], scalar2=rstd,
                    op0=OP.subtract, op1=OP.mult,
                )
            return xn

        def transpose_into(src_tile, qt, dst_tiles, g_c, b_c):
            """src natural [P, D] bf16 tile (token tile qt) -> dst [d,q] cols,
            applying the LN gain/bias per partition during the copyback."""
            for dt in range(ND):
                pt = psum.tile([P, P], MMD, tag="tp", name="tp", bufs=2)
                nc.tensor.transpose(pt, src_tile[:, ts(dt, P)], ident)
                nc.vector.tensor_scalar(
                    out=dst_tiles[dt][:, ts(qt, P)], in0=pt,
                    scalar1=g_c[:, dt:dt + 1], scalar2=b_c[:, dt:dt + 1],
                    op0=OP.mult, op1=OP.add,
                )

        # persistent activations (tag groups; A is reused by x2nT later)
        x1T = big_tiles([P, L], "A", "x1T", MMD)
        qT = big_tiles([P, L], "B", "qT", MMD)
        kT = big_tiles([P, L], "C", "kT", MMD)
        attnT = big_tiles([P, L], "AT", "attnT", MMD)
        vn = [
            big.tile([P, H, DK + 1], MMD, tag=f"V{i}", name=f"vn{i}", bufs=1)
            for i in range(NQ)
        ]
        x2 = big_tiles([P, D], "X2", "x2", FP32)

        # weight prefetch DMAs: emitted after the x loads so the casting
        # DMAs queue behind the input on HBM. wv first (V phase needs it
        # first), then wo.
        wv_rows = []
        for dt in range(ND):
            wt = ph2v.tile([P, D], MMD, tag=f"wv_row{dt}",
                           name=f"wv_row{dt}", bufs=1)
            nc.gpsimd.dma_start(out=wt, in_=wv.ap()[ts(dt, P), :])
            wv_rows.append(wt)
        def load_qk_w0(wmat, tag):
            wt = singles.tile([P, ND, P], MMD, name=tag)
            nc.gpsimd.dma_start(
                out=wt,
                in_=wmat.ap().rearrange("(a p) b -> p a b", p=P)[:, :, ts(0, P)],
            )
            return wt

        qk_pre0 = (load_qk_w0(wq, "wq0"), load_qk_w0(wk, "wk0"))
        wo_rows = []
        for dt in range(ND):
            wt = ph4w.tile([P, D], MMD, tag=f"wo_row{dt}",
                           name=f"wo_row{dt}", bufs=1)
            nc.gpsimd.dma_start(out=wt, in_=wo.ap()[ts(dt, P), :])
            wo_rows.append(wt)
        for qt in range(NQ):
            nc.vector.tensor_copy(out=vn[qt][:, :, DK:DK + 1], in_=ones_h)

        # ---------- phase 1: LN1 + transpose ----------
        with tc.tile_pool(name="ph1w", bufs=3) as ph1w:
            for qt in range(NQ):
                x1 = layer_norm_tile(ph1w, x_in[qt], use_act=True)
                transpose_into(x1, qt, x1T, g1_c, b1ln_c)

        # ---------- phase 2: V natural (+ones col) ----------
        # dt-major groups of 4 chains: the PE stream is in-order, so this
        # puts 28 early-ready matmuls ahead of the first MM that waits on
        # the late-arriving wv rows (instead of head-of-line blocking
        # after 7).
        units = [(qt, ch) for qt in range(NQ) for ch in range(NCH)]
        for g in range(0, len(units), 4):
            grp = units[g:g + 4]
            pss = [psum_mm() for _ in grp]
            for dt in range(ND):
                for ps, (qt, ch) in zip(pss, grp):
                    nc.tensor.matmul(
                        ps, x1T[dt][:, ts(qt, P)],
                        wv_rows[dt][:, ts(ch, CH)],
                        start=(dt == 0), stop=False,
                    )
            for ps, (qt, ch) in zip(pss, grp):
                # fold bv in as a K=1 broadcast matmul; drain on the
                # (idle during this phase) ACT engine
                nc.tensor.matmul(
                    ps, ones_row, bv_row[:, ts(ch, CH)],
                    start=False, stop=True,
                )
                nc.scalar.activation(
                    out=vn[qt][:, ds(ch * HPC, HPC), 0:DK],
                    in_=ps.rearrange("p (h d) -> p h d", d=DK),
                    func=AF.Identity,
                )
        ph1.release()

        # ---------- phases 3: QK + attention (chunk-major) ----------
        with tc.tile_pool(name="ph3", bufs=3) as ph3, \
             tc.tile_pool(name="ph3w", bufs=2) as ph3w:

            def emit_attention_pair_chunk(dt, ch):
                """S (row-packed across both heads of d-tile dt), exp, and
                the PV' accumulation step per k-tile, for token chunk ch.
                exp runs on ACT for even k-tiles and as a Schraudolph
                int16 bit-trick on DVE for odd k-tiles, so the two psum
                drains proceed in parallel."""
                heads = (2 * dt, 2 * dt + 1)
                # chunk 0 runs before any ph4 transposes: odd pairs borrow
                # the idle tp banks so consecutive pairs' PV accumulation
                # overlaps the previous pair's tail copies.
                if ch == 0 and dt % 2 == 1:
                    pa = {
                        h: psum.tile([P, CH], FP32, tag="tp",
                                     name="ps_a", bufs=2)
                        for h in heads
                    }
                else:
                    pa = {
                        h: psum.tile([P, CH], FP32, tag=f"pv{h % 2}",
                                     name="ps_a", bufs=1)
                        for h in heads
                    }
                # software-pipelined by one kt: the PE stream is in-order,
                # so S(kt+1) is emitted ahead of PV(kt) — by the time the PE
                # reaches PV(kt), its exp drain has had a full S-pair of
                # extra latency to finish.
                es_hist = {}
                for kt in range(NQ):
                    es = {}
                    for h in heads:
                        rbase = (h % 2) * DK
                        ps = psum_mm()
                        nc.tensor.matmul(
                            ps,
                            kT[dt][rbase:rbase + DK, ts(kt, P)],
                            qT[dt][rbase:rbase + DK, ts(ch, CH)],
                            start=True, stop=True,
                        )
                        if kt not in (1, 5):
                            e = ph3.tile([P, CH], MMD, tag=f"expS{h % 2}",
                                         name="expS", bufs=4)
                            nc.scalar.activation(
                                out=e, in_=ps, func=AF.Exp,
                                bias=ebias[:, kt:kt + 1], scale=0.125,
                            )
                        else:
                            ei = ph3.tile([P, CH], I16, tag=f"expI{h % 2}",
                                          name="expI", bufs=4)
                            nc.vector.tensor_scalar(
                                out=ei, in0=ps, scalar1=SCHRA_A * 0.125,
                                scalar2=eb16[:, kt:kt + 1],
                                op0=OP.mult, op1=OP.add,
                            )
                            e = ei.bitcast(MMD)
                        es[h] = e
                    es_hist[kt] = es
                    if kt >= 1:
                        for h in heads:
                            nc.tensor.matmul(
                                pa[h][0:DK + 1, :],
                                vn[kt - 1][:, h, :],
                                es_hist[kt - 1][h],
                                start=(kt - 1 == 0), stop=False,
                            )
                for h in heads:
                    nc.tensor.matmul(
                        pa[h][0:DK + 1, :],
                        vn[NQ - 1][:, h, :],
                        es_hist[NQ - 1][h],
                        start=False, stop=True,
                    )
                for h in heads:
                    rbase = (h % 2) * DK
                    # decouple the tail so the PV psum recycles after one copy
                    pv_sb = ph3.tile([P, CH], FP32, tag="pv_sb", name="pv_sb",
                                     bufs=4)
                    nc.vector.tensor_copy(out=pv_sb[0:DK, :],
                                          in_=pa[h][0:DK, :])
                    # Z row staged to partition 0 (partition_broadcast reads
                    # its input's first partition from a base-0 AP only),
                    # reciprocal'd in one DVE pass, replicated on gpsimd.
                    zrow = ph3.tile([1, CH], FP32, tag="zrow", name="zrow",
                                    bufs=1)
                    nc.vector.tensor_copy(out=zrow, in_=pa[h][DK:DK + 1, :])
                    rzrow = ph3.tile([1, CH], FP32, tag="rzrow", name="rzrow",
                                     bufs=1)
                    nc.vector.reciprocal_approx_fast(out=rzrow, in_=zrow)
                    rzb = ph3.tile([DK, CH], FP32, tag="rzb", name="rzb",
                                   bufs=1)
                    nc.gpsimd.partition_broadcast(rzb, rzrow)
                    nc.vector.tensor_mul(
                        out=attnT[dt][rbase:rbase + DK, ts(ch, CH)],
                        in0=pv_sb[0:DK, :], in1=rzb,
                    )

            for dt_out in range(ND):
                for wi, (wmat, bias_c, dstT) in enumerate(
                        ((wq, bq_c, qT), (wk, bk_c, kT))):
                    if dt_out == 0:
                        wt = qk_pre0[wi]
                    else:
                        wt = ph3w.tile([P, ND, P], MMD, tag="w_col",
                                       name="w_col", bufs=2)
                        nc.gpsimd.dma_start(
                            out=wt,
                            in_=wmat.ap().rearrange("(a p) b -> p a b", p=P)[
                                :, :, ts(dt_out, P)],
                        )
                    for ch in range(NCH):
                        ps = psum_mm()
                        for dt_in in range(ND):
                            nc.tensor.matmul(
                                ps, wt[:, dt_in, :],
                                x1T[dt_in][:, ts(ch, CH)],
                                start=(dt_in == 0), stop=(dt_in == ND - 1),
                            )
                        nc.scalar.activation(
                            out=dstT[dt_out][:, ts(ch, CH)], in_=ps,
                            func=AF.Identity, bias=bias_c[:, dt_out:dt_out + 1],
                            scale=1.0,
                        )
                emit_attention_pair_chunk(dt_out, 0)
            for dt_out in range(ND):
                emit_attention_pair_chunk(dt_out, 1)

            # ---------- phase 4+5: out-proj + residual + LN2 + transpose ----
            # Emitted inside the ph3 scope, chunk-major, so chunk-0 proj/LN2
            # overlaps the chunk-1 attention still in flight.
            x2nT = big_tiles([P, L], "A", "x2nT", MMD)  # reuses x1T slots
            with tc.tile_pool(name="ph4", bufs=2) as ph4:
                for ch in range(NCH):
                    for qi in range(QPC):
                        qt = ch * QPC + qi
                        x_t = ph4.tile([P, D], FP32, tag="x_again", name="x_again")
                        nc.sync.dma_start(out=x_t, in_=xd.ap()[ts(qt, P), :])
                        for oc in range(NCH):
                            ps = psum_mm()
                            for dt in range(ND):
                                nc.tensor.matmul(
                                    ps, attnT[dt][:, ts(qt, P)],
                                    wo_rows[dt][:, ts(oc, CH)],
                                    start=(dt == 0), stop=False,
                                )
                            # fold bo in as a K=1 broadcast matmul
                            nc.tensor.matmul(
                                ps, ones_row, bo_row[:, ts(oc, CH)],
                                start=False, stop=True,
                            )
                            nc.vector.tensor_add(
                                out=x2[qt][:, ts(oc, CH)], in0=ps,
                                in1=x_t[:, ts(oc, CH)],
                            )
                        x2n = layer_norm_tile(ph4, x2[qt])
                        transpose_into(x2n, qt, x2nT, g2_c, b2ln_c)

        # ---------- phase 6: FFN (two halves of 16 f-tiles) ----------
        # acc lives as [P, CH] chunks in the dead attnT (oc=0) / kT (oc=1)
        # tag slots; vn's V tags shrink back to vn size.
        acc_t = {}
        for qt in range(NQ):
            acc_t[(qt, 0)] = big.tile([P, CH], FP32, tag=f"AT{qt}",
                                      name=f"acc{qt}_0", bufs=1)
            acc_t[(qt, 1)] = big.tile([P, CH], FP32, tag=f"C{qt}",
                                      name=f"acc{qt}_1", bufs=1)

        with tc.tile_pool(name="ph6w", bufs=2) as ph6w:
            w1r = w1.ap().rearrange("(a p) b -> p a b", p=P)
            for half in range(2):
                hts = []
                w2_rows = []
                for fi in range(NHALF):
                    ft = half * NHALF + fi
                    w1t = ph6w.tile([P, ND, P], MMD, tag="w1_col",
                                    name="w1_col", bufs=6)
                    nc.gpsimd.dma_start(out=w1t, in_=w1r[:, :, ts(ft, P)])
                    if fi < ND:
                        w2t = ph4w.tile([P, D], MMD, tag=f"wo_row{fi}",
                                        name=f"w2_row{ft}", bufs=1)
                    else:
                        w2t = ph2v.tile([P, D], MMD, tag=f"wv_row{fi - ND}",
                                        name=f"w2_row{ft}", bufs=1)
                    nc.gpsimd.dma_start(out=w2t, in_=w2.ap()[ts(ft, P), :])
                    w2_rows.append(w2t)
                    httag = f"B{fi}" if fi < ND else f"V{fi - ND}"
                    ht = big.tile([P, L], MMD, tag=httag,
                                  name=f"h{half}t{fi}", bufs=1)
                    for ch in range(NCH):
                        ps = psum_mm()
                        for dt in range(ND):
                            nc.tensor.matmul(
                                ps, w1t[:, dt, :],
                                x2nT[dt][:, ts(ch, CH)],
                                start=(dt == 0), stop=(dt == ND - 1),
                            )
                        nc.scalar.activation(
                            out=ht[:, ts(ch, CH)], in_=ps, func=AF.Relu,
                            bias=b1_c[:, ft:ft + 1], scale=1.0,
                        )
                    hts.append(ht)
                for qt in range(NQ):
                    for oc in range(NCH):
                        ps = psum_mm()
                        for fi in range(NHALF):
                            nc.tensor.matmul(
                                ps, hts[fi][:, ts(qt, P)],
                                w2_rows[fi][:, ts(oc, CH)],
                                start=(fi == 0),
                                stop=(fi == NHALF - 1 and half == 1),
                            )
                        if half == 0:
                            # fold the fc2 bias in as a K=1 broadcast matmul
                            nc.tensor.matmul(
                                ps, ones_row, b2_row[:, ts(oc, CH)],
                                start=False, stop=True,
                            )
                            # and the residual stream via the copy-out add
                            nc.vector.tensor_add(
                                out=acc_t[(qt, oc)],
                                in0=ps, in1=x2[qt][:, ts(oc, CH)],
                            )
                        else:
                            nc.vector.tensor_add(
                                out=acc_t[(qt, oc)],
                                in0=acc_t[(qt, oc)], in1=ps,
                            )
                            seng = nc.sync if qt % 2 == 0 else nc.scalar
                            seng.dma_start(
                                out=outd.ap()[ts(qt, P), ts(oc, CH)],
                                in_=acc_t[(qt, oc)],
                            )

        ph4w.release()
        ph2v.release()
        psum.release()
        big.release()
        singles.release()

    nc.finalize()
    return nc


_NC_CACHE = None


def _get_nc():
    global _NC_CACHE
    if _NC_CACHE is None:
        _NC_CACHE = build_nc()
    return _NC_CACHE


def run(inputs, trace=False):
    """Run on 8 cores; returns (out [8,L,D], BassKernelResults)."""
    from concourse.bass_utils import run_bass_kernel_spmd

    nc = _get_nc()
    weights = {
        k: np.ascontiguousarray(np.asarray(inputs[k], dtype=np.float32))
        for k in ("ln1_g", "ln1_b", "Wq", "bq", "Wk", "bk", "Wv", "bv",
                  "Wo", "bo", "ln2_g", "ln2_b", "W1", "b1", "W2", "b2")
    }
    x = np.asarray(inputs["x"], dtype=np.float32)
    e_mask = np.asarray(inputs["e_mask"], dtype=np.int32)
    in_maps = []
    for b in range(B):
        m = dict(weights)
        m["x"] = np.ascontiguousarray(x[b])
        m["e_mask"] = np.ascontiguousarray(e_mask[b])
        in_maps.append(m)
    import time as _time

    last_err = None
    for _attempt in range(5):
        try:
            res = run_bass_kernel_spmd(
                nc, in_maps, core_ids=list(range(B)), trace=trace)
            break
        except Exception as e:  # transient NRT_EXEC_UNIT_UNRECOVERABLE wedges
            last_err = e
            _time.sleep(2.0 * (_attempt + 1))  # let the device session recover
    else:
        raise last_err
    out = np.stack([res.results[b]["out"] for b in range(B)], axis=0)
    return out, res


def kernel(**inputs):
    out, _ = run(inputs, trace=False)
    return out
